# revision 13
# baseline (speedup 1.0000x reference)
"""DCNv3 Trainium2 kernel: 8-core SPMD, core = (batch, group-pair).

Wire-minimal design for the axon tunnel (~45 MB/s H2D, ~28 MB/s D2H):
  - per-core inputs: xin [32, H*W] bf16 (the core's OWN 32 channels only),
    cstb/cstf consts. An in-kernel AllGather over pairs [[0,1],[2,3],...]
    reconstructs all 64 channels for the offset/mask conv on device.
  - on-device prologue rebuilds zero-padded grids (xfpo 64ch, xfpw 32ch)
    in DRAM.
  - per chunk: om = w_om @ x (PE); clamped-tri fields; A9'[(g,k),t] =
    sigmoid(ml)*ay_m*ax_n; A9' bounced through DRAM and broadcast 1->16
    partitions per (g,k) (k-major rows k*16+ch); x shifted copies DMA'd
    straight from xfp DRAM (plain slices); val = sum_t A9r * x_shift (DVE);
    conv via PE; GroupNorm + exact Gelu; outv in bf16.
  - dispatch: jit(shard_map(bass_exec)) built ONCE and cached; donated
    output buffers are created on-device by a tiny jitted zeros fn (no H2D).
    Mirrors bass_utils.run_bass_kernel_spmd's axon path (bass2jax); falls
    back to run_bass_kernel_spmd when not under axon.
Exact for |offset| <= 1; device outputs max|offset| and the host applies an
exact numpy correction for the (rare) larger offsets.
"""
import sys
import numpy as np
from contextlib import ExitStack

for _p in ("/opt/trn_rl_repo",):
    if _p not in sys.path:
        sys.path.insert(0, _p)

G, K, CG, H, W = 4, 9, 16, 128, 128
HP, WP = H + 4, W + 4
PX = HP * WP            # 17424
NPIX = H * W            # 16384
TS = WP + 1             # 133: tap-shift slack inside xr windows
MARG = 2 * WP + 8       # 272: margin so all shifted reads stay in-bounds
PXP = PX + 2 * MARG     # 17968
CHUNK = 484             # divides PX (36 chunks)
NCH = PX // CHUNK       # 36
CW = CHUNK + 2 * TS     # 750
EPS = 1e-5
NCORES = 8

_CACHE = {}


def _build_nc(mdt_name):
    import concourse.mybir as mybir
    from concourse import bacc, tile

    f32 = mybir.dt.float32
    mdt = getattr(mybir.dt, mdt_name)
    AF = mybir.ActivationFunctionType
    OP = mybir.AluOpType
    AX = mybir.AxisListType

    nc = bacc.Bacc("TRN2", target_bir_lowering=False, debug=False,
                   num_devices=NCORES)
    xin = nc.dram_tensor("xin", [32, NPIX], mdt, kind="ExternalInput")
    cstb = nc.dram_tensor("cstb", [128, 146], mdt, kind="ExternalInput")
    cstf = nc.dram_tensor("cstf", [36, 8], f32, kind="ExternalInput")
    outv = nc.dram_tensor("outv", [32, NPIX], mdt, kind="ExternalOutput")
    statso = nc.dram_tensor("statso", [32, 2], f32, kind="ExternalOutput")
    moffo = nc.dram_tensor("moffo", [36, 1], f32, kind="ExternalOutput")

    with ExitStack() as ctx:
        tc = ctx.enter_context(tile.TileContext(nc))
        cpool = ctx.enter_context(tc.tile_pool(name="consts", bufs=1))
        ppool = ctx.enter_context(tc.tile_pool(name="psum", bufs=4, space="PSUM"))
        dpool = ctx.enter_context(tc.tile_pool(name="drsc", bufs=1, space="DRAM"))
        keep = ctx.enter_context(tc.tile_pool(name="keep", bufs=1))

        sb_cb = cpool.tile([128, 146], mdt)
        nc.sync.dma_start(sb_cb[:], cstb[:])
        sb_cf = cpool.tile([36, 8], f32)
        nc.sync.dma_start(sb_cf[:], cstf[:])
        sb_womT = sb_cb[0:64, 0:82]
        sb_wA = [sb_cb[:, 82:98], sb_cb[:, 98:114]]       # rows (k*16+ci)
        sb_wB = [sb_cb[0:16, 114:130], sb_cb[0:16, 130:146]]  # rows ci
        sb_bomYX = sb_cf[0:36, 0:1]
        sb_bomM = sb_cf[0:18, 1:2]
        sb_dcnb = [sb_cf[0:16, 2:3], sb_cf[0:16, 3:4]]
        sb_gnwf = [sb_cf[0:16, 4:5], sb_cf[0:16, 5:6]]
        sb_gnbf = [sb_cf[0:16, 6:7], sb_cf[0:16, 7:8]]

        vsb = [keep.tile([16, PX], mdt, tag=f"vsb{g}", name=f"vsb{g}")
               for g in range(2)]
        moffa = keep.tile([36, 1], f32)
        nc.vector.memset(moffa[:], 0.0)

        # ---- prologue: pair AllGather x, rebuild padded grids in DRAM ----
        # xfpo: gathered 64ch grid (om input, original channel order)
        # xfpw: own 32ch grid (shifted-window source)
        xfpo = dpool.tile([64, PXP], mdt, tag="xfpo", name="xfpo")
        xfpw = dpool.tile([32, PXP], mdt, tag="xfpw", name="xfpw")
        xb = dpool.tile([32, NPIX], mdt, tag="xb", name="xb")
        xg = dpool.tile([64, NPIX], mdt, tag="xg", name="xg")
        ZW = PXP // 4   # 4492
        with tc.tile_pool(name="pre", bufs=1) as pre:
            z64 = pre.tile([64, ZW], mdt)
            nc.vector.memset(z64[:], 0.0)
            for j in range(4):
                nc.sync.dma_start(xfpo[:, j * ZW:(j + 1) * ZW], z64[:])
                nc.sync.dma_start(xfpw[:, j * ZW:(j + 1) * ZW], z64[0:32, :])
            nc.gpsimd.dma_start(xb[:], xin[:])
            nc.gpsimd.collective_compute(
                "AllGather", mybir.AluOpType.bypass,
                replica_groups=[[0, 1], [2, 3], [4, 5], [6, 7]],
                ins=[xb[:].opt()], outs=[xg[:].opt()])
            xfpo_in = xfpo[:, MARG:MARG + PX].rearrange(
                "p (h w) -> p h w", w=WP)[:, 2:2 + H, 2:2 + W]
            nc.sync.dma_start(
                xfpo_in, xg[:].rearrange("p (h w) -> p h w", w=W))
            xfpw_in = xfpw[:, MARG:MARG + PX].rearrange(
                "p (h w) -> p h w", w=WP)[:, 2:2 + H, 2:2 + W]
            nc.sync.dma_start(
                xfpw_in, xin[:].rearrange("p (h w) -> p h w", w=W))

        # ----- fused per-chunk pipeline -----
        with tc.tile_pool(name="p2", bufs=2) as p2, \
             tc.tile_pool(name="pbig", bufs=1) as pbig, \
             tc.tile_pool(name="dscr", bufs=2, space="DRAM") as dscr:
            for c in range(NCH):
                q = c * CHUNK
                xc = p2.tile([64, CHUNK], mdt, tag="xc")
                nc.sync.dma_start(xc[:], xfpo[:, MARG + q:MARG + q + CHUNK])
                omYX = p2.tile([36, CHUNK], f32, tag="omYX")
                omM = p2.tile([18, CHUNK], f32, tag="omM")
                ps = ppool.tile([82, CHUNK], f32, tag="omps")
                nc.tensor.matmul(ps[:], sb_womT, xc[:], start=True, stop=True)
                nc.scalar.activation(omYX[:], ps[0:36, :],
                                     AF.Identity, bias=sb_bomYX)
                nc.scalar.activation(omM[:], ps[64:82, :],
                                     AF.Identity, bias=sb_bomM)
                ayx = p2.tile([36, 3, CHUNK], mdt, tag="ayx")
                for m in range(3):
                    tmp = p2.tile([36, CHUNK], f32, tag="tmp_m")
                    tabs = p2.tile([36, CHUNK], f32, tag="tabs_m")
                    nc.vector.tensor_scalar(tmp[:], omYX[:], float(1 - m),
                                            None, OP.add)
                    nc.vector.scalar_tensor_tensor(tabs[:], tmp[:], -1.0,
                                                   tmp[:], OP.mult, OP.max)
                    if m == 1:
                        mr = p2.tile([36, 1], f32, tag="mr")
                        nc.vector.tensor_reduce(mr[:], tabs[:], axis=AX.X,
                                                op=OP.max)
                        nc.vector.tensor_tensor(moffa[:], moffa[:], mr[:],
                                                OP.max)
                    nc.scalar.activation(ayx[:, m, :], tabs[:], AF.Relu,
                                         bias=1.0, scale=-1.0)
                ms = p2.tile([18, CHUNK], mdt, tag="ms")
                nc.scalar.activation(ms[:], omM[:], AF.Sigmoid)
                axT = p2.tile([18, 3, CHUNK], mdt, tag="axT")
                nc.sync.dma_start(axT[:], ayx[18:36, :, :])
                ayp = p2.tile([18, 3, CHUNK], mdt, tag="ayp")
                for m in range(3):
                    nc.vector.tensor_tensor(ayp[:, m, :], ayx[0:18, m, :],
                                            ms[:], OP.mult)
                a9p = p2.tile([18, K, CHUNK], mdt, tag="a9p")
                for t in range(K):
                    m, n = t // 3, t % 3
                    nc.vector.tensor_tensor(a9p[:, t, :], ayp[:, m, :],
                                            axT[:, n, :], OP.mult)
                # bounce A9' through DRAM, broadcast 1 row -> 16 partitions
                a9d = dscr.tile([18, K, CHUNK], mdt, tag="a9d")
                nc.sync.dma_start(a9d[:], a9p[:])
                a9rA = [pbig.tile([128, K, CHUNK], mdt, tag=f"a9rA{g}",
                                  name=f"a9rA{g}") for g in range(2)]
                a9rB = [pbig.tile([16, K, CHUNK], mdt, tag=f"a9rB{g}",
                                  name=f"a9rB{g}") for g in range(2)]
                for g in range(2):
                    for k in range(8):
                        nc.sync.dma_start(
                            a9rA[g][k * 16:(k + 1) * 16, :, :],
                            a9d[g * 9 + k:g * 9 + k + 1, :, :]
                            .to_broadcast((16, K, CHUNK)))
                    nc.sync.dma_start(
                        a9rB[g][:],
                        a9d[g * 9 + 8:g * 9 + 9, :, :]
                        .to_broadcast((16, K, CHUNK)))
                # shifted x windows straight from DRAM (k-major rows)
                xrA = [p2.tile([128, CW], mdt, tag=f"xrA{g}",
                               name=f"xrA{g}") for g in range(2)]
                xrB = [p2.tile([16, CW], mdt, tag=f"xrB{g}",
                               name=f"xrB{g}") for g in range(2)]
                for g in range(2):
                    r0 = 16 * g
                    for k in range(8):
                        soff = (k // 3 - 1) * WP + (k % 3 - 1)
                        c0 = MARG + q + soff - TS
                        nc.sync.dma_start(xrA[g][k * 16:(k + 1) * 16, :],
                                          xfpw[r0:r0 + 16, c0:c0 + CW])
                    c0 = MARG + q + (WP + 1) - TS
                    nc.sync.dma_start(xrB[g][:],
                                      xfpw[r0:r0 + 16, c0:c0 + CW])
                # modulation: val = sum_t A9r * x_shift
                val = [pbig.tile([128, CHUNK], mdt, tag=f"val{i}",
                                 name=f"val{i}") for i in range(2)]
                valB = [pbig.tile([16, CHUNK], mdt, tag=f"valB{g}",
                                  name=f"valB{g}") for g in range(2)]
                tiles = [(val[0], a9rA[0], xrA[0], "pA0"),
                         (val[1], a9rA[1], xrA[1], "pA1"),
                         (valB[0], a9rB[0], xrB[0], "pB0"),
                         (valB[1], a9rB[1], xrB[1], "pB1")]
                for vt, ar, xr, ptag in tiles:
                    rows = vt.shape[0]
                    prod = pbig.tile([rows, CHUNK], mdt, tag=ptag, name=ptag)
                    for t in range(K):
                        m, n = t // 3, t % 3
                        off = TS + (m - 1) * WP + (n - 1)
                        dst = vt if t == 0 else prod
                        nc.vector.tensor_tensor(dst[:], ar[:, t, :],
                                                xr[:, off:off + CHUNK],
                                                OP.mult)
                        if t > 0:
                            nc.vector.tensor_tensor(vt[:], vt[:], prod[:],
                                                    OP.add)
                # conv
                for g in range(2):
                    psv = ppool.tile([16, CHUNK], f32, tag="psv")
                    nc.tensor.matmul(psv[:], sb_wA[g], val[g][:],
                                     start=True, stop=False)
                    nc.tensor.matmul(psv[:], sb_wB[g], valB[g][:],
                                     start=False, stop=True)
                    nc.scalar.activation(vsb[g][:, q:q + CHUNK],
                                         psv[:], AF.Identity, bias=sb_dcnb[g])
        nc.sync.dma_start(moffo[:], moffa[:])

        # ---------------- GroupNorm + Gelu ----------------
        with tc.tile_pool(name="p3", bufs=1) as p3:
            VOFF = 2 * WP
            invN = 1.0 / (CG * NPIX)
            zero16 = p3.tile([1, 16], f32)
            nc.vector.memset(zero16[:], 0.0)
            for g in range(2):
                vg = vsb[g]
                vap = vg[:, VOFF:VOFF + H * WP].rearrange(
                    "p (h w) -> p h w", w=WP)[:, :, 2:2 + W]
                vsq = p3.tile([16, PX], f32, tag="vsq", name="vsq")
                nc.scalar.activation(vsq[:], vg[:], AF.Square)
                sqap = vsq[:, VOFF:VOFF + H * WP].rearrange(
                    "p (h w) -> p h w", w=WP)[:, :, 2:2 + W]
                r1 = p3.tile([16, H], f32, tag="r1", name="r1")
                s1 = p3.tile([16, 1], f32, tag="s1", name="s1")
                nc.vector.tensor_reduce(r1[:], vap, axis=AX.X, op=OP.add)
                nc.vector.tensor_reduce(s1[:], r1[:], axis=AX.X, op=OP.add)
                r2 = p3.tile([16, H], f32, tag="r2", name="r2")
                s2 = p3.tile([16, 1], f32, tag="s2", name="s2")
                nc.vector.tensor_reduce(r2[:], sqap, axis=AX.X, op=OP.add)
                nc.vector.tensor_reduce(s2[:], r2[:], axis=AX.X, op=OP.add)
                stats = p3.tile([16, 2], f32, tag="stats", name="stats")
                nc.vector.tensor_copy(stats[:, 0:1], s1[:])
                nc.vector.tensor_copy(stats[:, 1:2], s2[:])
                nc.sync.dma_start(statso[g * 16:g * 16 + 16, :], stats[:])
                scr1 = dpool.tile([16, 1], f32, tag="scr1", name="scr1")
                scr2 = dpool.tile([16, 1], f32, tag="scr2", name="scr2")
                nc.sync.dma_start(scr1[:], s1[:])
                nc.sync.dma_start(scr2[:], s2[:])
                s1t = p3.tile([1, 16], f32, tag="s1t", name="s1t")
                s2t = p3.tile([1, 16], f32, tag="s2t", name="s2t")
                nc.sync.dma_start(s1t[:], scr1[:].rearrange("p x -> x p"))
                nc.sync.dma_start(s2t[:], scr2[:].rearrange("p x -> x p"))
                mug = p3.tile([1, 1], f32, tag="mug", name="mug")
                e2g = p3.tile([1, 1], f32, tag="e2g", name="e2g")
                nc.vector.tensor_reduce(mug[:], s1t[:], axis=AX.X, op=OP.add)
                nc.vector.tensor_reduce(e2g[:], s2t[:], axis=AX.X, op=OP.add)
                nc.vector.tensor_scalar(mug[:], mug[:], invN, None, OP.mult)
                nc.vector.tensor_scalar(e2g[:], e2g[:], invN, None, OP.mult)
                var = p3.tile([1, 1], f32, tag="var", name="var")
                nc.vector.tensor_tensor(var[:], mug[:], mug[:], OP.mult)
                nc.vector.tensor_tensor(var[:], e2g[:], var[:], OP.subtract)
                nc.vector.tensor_scalar(var[:], var[:], EPS, None, OP.add)
                sd = p3.tile([1, 1], f32, tag="sd", name="sd")
                nc.scalar.activation(sd[:], var[:], AF.Sqrt)
                ivg = p3.tile([1, 1], f32, tag="ivg", name="ivg")
                nc.vector.reciprocal(ivg[:], sd[:])
                inv16 = p3.tile([1, 16], f32, tag="inv16", name="inv16")
                mu16 = p3.tile([1, 16], f32, tag="mu16", name="mu16")
                nc.scalar.activation(inv16[:], zero16[:], AF.Identity,
                                     bias=ivg[:])
                nc.scalar.activation(mu16[:], zero16[:], AF.Identity,
                                     bias=mug[:])
                ivp = p3.tile([16, 1], f32, tag="ivp", name="ivp")
                mup = p3.tile([16, 1], f32, tag="mup", name="mup")
                scr3 = dpool.tile([1, 16], f32, tag="scr3", name="scr3")
                scr4 = dpool.tile([1, 16], f32, tag="scr4", name="scr4")
                nc.sync.dma_start(scr3[:], inv16[:])
                nc.sync.dma_start(scr4[:], mu16[:])
                nc.sync.dma_start(ivp[:], scr3[:].rearrange("x p -> p x"))
                nc.sync.dma_start(mup[:], scr4[:].rearrange("x p -> p x"))
                scp = p3.tile([16, 1], f32, tag="scp", name="scp")
                bip = p3.tile([16, 1], f32, tag="bip", name="bip")
                nc.vector.tensor_tensor(scp[:], sb_gnwf[g], ivp[:], OP.mult)
                nc.vector.tensor_tensor(bip[:], mup[:], scp[:], OP.mult)
                nc.vector.tensor_tensor(bip[:], sb_gnbf[g], bip[:],
                                        OP.subtract)
                og = p3.tile([16, PX], mdt, tag="og", name="og")
                nc.scalar.activation(og[:], vg[:], AF.Gelu,
                                     bias=bip[:], scale=scp[:])
                ogap = og[:, VOFF:VOFF + H * WP].rearrange(
                    "p (h w) -> p h w", w=WP)[:, :, 2:2 + W]
                nc.sync.dma_start(
                    outv[g * 16:g * 16 + 16, :].rearrange(
                        "p (h w) -> p h w", w=W),
                    ogap)

    if not nc.is_finalized():
        nc.finalize()
    return nc


def get_nc(mdt_name="bfloat16"):
    key = ("nc", mdt_name)
    if key not in _CACHE:
        _CACHE[key] = _build_nc(mdt_name)
    return _CACHE[key]


def _host_prep(x, w_om, b_om, dcn_w, dcn_b, gn_w, gn_b, offset_scale, cast):
    B = x.shape[0]
    sc = float(np.asarray(offset_scale).reshape(-1)[0])
    idx_oy = [g * 27 + 2 * k for g in range(G) for k in range(K)]
    idx_ox = [g * 27 + 2 * k + 1 for g in range(G) for k in range(K)]
    idx_ml = [g * 27 + 18 + k for g in range(G) for k in range(K)]
    xb = np.ascontiguousarray(x.reshape(B, 64, NPIX)).astype(cast)
    in_maps = []
    for core in range(NCORES):
        b, gp = core // 2, core % 2
        gsel = [2 * gp, 2 * gp + 1]
        own = slice(gsel[0] * CG, gsel[0] * CG + 2 * CG)
        m = {"xin": xb[b][own]}
        cols = []
        for idx in (idx_oy, idx_ox, idx_ml):
            for g in gsel:
                cols += idx[g * K:(g + 1) * K]
        wsel = w_om[cols].astype(np.float32).copy()
        bsel = b_om[cols].astype(np.float32).copy()
        wsel[:36] *= sc
        bsel[:36] *= sc
        cb = np.zeros((128, 146), np.float32)
        cb[0:64, 0:82] = np.concatenate(
            [wsel[0:36], np.zeros((28, 64), np.float32), wsel[36:54]]).T
        for gi in range(2):
            wg = dcn_w[gsel[gi]].reshape(CG, CG, K)   # [co, ci, k]
            # k-major rows (k*16+ci), cols co
            cb[:, 82 + 16 * gi:98 + 16 * gi] = np.transpose(
                wg[:, :, :8], (2, 1, 0)).reshape(128, CG)
            cb[0:16, 114 + 16 * gi:130 + 16 * gi] = wg[:, :, 8].T
        m["cstb"] = cb.astype(cast)
        cf = np.zeros((36, 8), np.float32)
        cf[0:36, 0] = bsel[0:36]
        cf[0:18, 1] = bsel[36:54]
        cf[0:16, 2] = dcn_b[gsel[0]]
        cf[0:16, 3] = dcn_b[gsel[1]]
        c0 = gsel[0] * CG
        cf[0:16, 4] = gn_w[c0:c0 + 16]
        cf[0:16, 5] = gn_w[c0 + 16:c0 + 32]
        cf[0:16, 6] = gn_b[c0:c0 + 16]
        cf[0:16, 7] = gn_b[c0 + 16:c0 + 32]
        m["cstf"] = cf
        in_maps.append(m)
    return in_maps


# ---------------- cached PJRT dispatch (axon path) ----------------

def _get_exec(mdt_name="bfloat16"):
    key = ("exec", mdt_name)
    if key in _CACHE:
        return _CACHE[key]
    import jax
    import jax.numpy as jnp
    from jax.sharding import Mesh, PartitionSpec, NamedSharding
    from jax.experimental.shard_map import shard_map
    from concourse import bass2jax
    import concourse.mybir as mybir

    nc = get_nc(mdt_name)
    bass2jax.install_neuronx_cc_hook()
    partition_name = (nc.partition_id_tensor.name
                      if nc.partition_id_tensor else None)
    in_names, out_names, out_avals, out_np = [], [], [], []
    for alloc in nc.m.functions[0].allocations:
        if not isinstance(alloc, mybir.MemoryLocationSet):
            continue
        name = alloc.memorylocations[0].name
        if alloc.kind == "ExternalInput":
            if name != partition_name:
                in_names.append(name)
        elif alloc.kind == "ExternalOutput":
            shape = tuple(alloc.tensor_shape)
            dtype = mybir.dt.np(alloc.dtype)
            out_names.append(name)
            out_avals.append(jax.core.ShapedArray(shape, dtype))
            out_np.append((shape, dtype))
    n_params = len(in_names)
    n_outs = len(out_names)
    in_names_all = list(in_names) + list(out_names)
    if partition_name is not None:
        in_names_all.append(partition_name)
    donate = tuple(range(n_params, n_params + n_outs))

    def _body(*args):
        operands = list(args)
        if partition_name is not None:
            operands.append(bass2jax.partition_id_tensor())
        outs = bass2jax._bass_exec_p.bind(
            *operands,
            out_avals=tuple(out_avals),
            in_names=tuple(in_names_all),
            out_names=tuple(out_names),
            lowering_input_output_aliases=(),
            sim_require_finite=True,
            sim_require_nnan=True,
            nc=nc,
        )
        return tuple(outs)

    devices = jax.devices()[:NCORES]
    mesh = Mesh(np.asarray(devices), ("core",))
    pspec = PartitionSpec("core")
    in_specs = (pspec,) * (n_params + n_outs)
    out_specs = (pspec,) * n_outs
    sharded = jax.jit(
        shard_map(_body, mesh=mesh, in_specs=in_specs, out_specs=out_specs,
                  check_rep=False),
        donate_argnums=donate, keep_unused=True)
    sh = NamedSharding(mesh, pspec)

    zdtypes = []
    import ml_dtypes
    for s, d in out_np:
        zdtypes.append(jnp.bfloat16 if d == ml_dtypes.bfloat16 else d)

    def _mk_zeros():
        return tuple(
            jnp.zeros((NCORES * s[0],) + tuple(s[1:]), zd)
            for (s, d), zd in zip(out_np, zdtypes))

    zeros_fn = jax.jit(_mk_zeros, out_shardings=(sh,) * n_outs)
    E = dict(sharded=sharded, zeros_fn=zeros_fn, in_names=in_names,
             out_names=out_names, out_np=out_np)
    _CACHE[key] = E
    return E


def _dispatch(in_maps, mdt_name="bfloat16"):
    """concat per-core maps, run on 8 cores, return per-core result dicts.

    Includes host concat + on-device zero-output creation + execute + fetch:
    the same work run_bass_kernel_spmd would do per call, with the jit cached.
    """
    E = _get_exec(mdt_name)
    concat_in = [np.concatenate([m[name] for m in in_maps], axis=0)
                 for name in E["in_names"]]
    zeros = _CACHE.pop(("zeros_next", mdt_name), None) or E["zeros_fn"]()
    out_arrs = E["sharded"](*concat_in, *zeros)
    # prefetch donated buffers for the NEXT call while this one runs
    _CACHE[("zeros_next", mdt_name)] = E["zeros_fn"]()
    for a in out_arrs:
        for s_ in a.addressable_shards:
            s_.data.copy_to_host_async()
    outs = [np.asarray(a) for a in out_arrs]
    results = []
    for c in range(NCORES):
        r = {}
        for i, name in enumerate(E["out_names"]):
            s0 = E["out_np"][i][0][0]
            r[name] = outs[i][c * s0:(c + 1) * s0]
        results.append(r)
    return results


def _run_spmd(nc, in_maps):
    """Fallback: stock dispatcher (non-axon environments)."""
    from concourse.bass_utils import run_bass_kernel_spmd
    res = run_bass_kernel_spmd(nc, in_maps, core_ids=list(range(NCORES)))
    return res.results


def kernel(x, w_om, b_om, dcn_w, dcn_b, gn_w, gn_b, offset_scale,
           _mdt="bfloat16"):
    import ml_dtypes

    x = np.asarray(x, np.float32)
    w_om = np.asarray(w_om, np.float32)
    b_om = np.asarray(b_om, np.float32)
    dcn_w = np.asarray(dcn_w, np.float32)
    dcn_b = np.asarray(dcn_b, np.float32)
    gn_w = np.asarray(gn_w, np.float32)
    gn_b = np.asarray(gn_b, np.float32)
    offset_scale = np.asarray(offset_scale, np.float32)
    cast = ml_dtypes.bfloat16 if _mdt == "bfloat16" else np.float32
    in_maps = _host_prep(x, w_om, b_om, dcn_w, dcn_b, gn_w, gn_b,
                         offset_scale, cast)
    try:
        from concourse.bass_utils import axon_active
        use_fast = axon_active()
    except Exception:
        use_fast = False
    if use_fast:
        results = _dispatch(in_maps, _mdt)
    else:
        results = _run_spmd(get_nc(_mdt), in_maps)
    out = np.zeros((4, 64, H, W), np.float32)
    stats = np.zeros((8, 32, 2), np.float32)
    moff_all = 0.0
    for core in range(NCORES):
        b, gp = core // 2, core % 2
        r = results[core]
        out[b, gp * 32:gp * 32 + 32] = np.asarray(
            r["outv"], np.float32).reshape(32, H, W)
        stats[core] = r["statso"]
        moff_all = max(moff_all, float(np.max(r["moffo"])))
    if moff_all > 1.0:
        out = _host_correct(out, stats, x, w_om, b_om, dcn_w, dcn_b,
                            gn_w, gn_b, offset_scale)
    return out


def _host_correct(out, stats, x, w_om, b_om, dcn_w, dcn_b, gn_w, gn_b,
                  offset_scale):
    """Exact fix for rare pixels with |offset| > 1 (clamped-tri mismatch)."""
    from scipy.special import erf, expit
    sc = float(np.asarray(offset_scale).reshape(-1)[0])
    B = x.shape[0]
    om = (np.einsum('bcp,oc->bop', x.reshape(B, 64, NPIX), w_om)
          + b_om[None, :, None]).reshape(B, 108, H, W)
    for b in range(B):
        for g in range(G):
            oy = om[b, g * 27:g * 27 + 18:2] * sc
            ox = om[b, g * 27 + 1:g * 27 + 18:2] * sc
            bad = (np.abs(oy) > 1).any(0) | (np.abs(ox) > 1).any(0)
            if not bad.any():
                continue
            ml = expit(om[b, g * 27 + 18:g * 27 + 27])
            core = b * 2 + g // 2
            gl = (g % 2) * 16
            N = CG * NPIX
            mu = stats[core, gl:gl + 16, 0].sum() / N
            var = stats[core, gl:gl + 16, 1].sum() / N - mu * mu
            inv = 1.0 / np.sqrt(var + EPS)
            wg = dcn_w[g].reshape(CG, CG, K)
            for hh, ww in zip(*np.nonzero(bad)):
                val = np.zeros((CG, K), np.float32)
                for k in range(K):
                    ky, kx = k // 3, k % 3
                    py = hh + ky - 1 + oy[k, hh, ww]
                    pxx = ww + kx - 1 + ox[k, hh, ww]
                    y0, x0 = int(np.floor(py)), int(np.floor(pxx))
                    fy, fx = py - y0, pxx - x0
                    acc = np.zeros(CG, np.float32)
                    for dy, wy in ((0, 1 - fy), (1, fy)):
                        for dx, wx in ((0, 1 - fx), (1, fx)):
                            yy, xx = y0 + dy, x0 + dx
                            if 0 <= yy < H and 0 <= xx < W:
                                acc += wy * wx * x[b, g * CG:g * CG + CG,
                                                   yy, xx]
                    val[:, k] = acc * ml[k, hh, ww]
                pre = np.einsum('ck,ock->o', val, wg) + dcn_b[g]
                z = ((pre - mu) * inv * gn_w[g * CG:g * CG + CG]
                     + gn_b[g * CG:g * CG + CG])
                out[b, g * CG:g * CG + CG, hh, ww] = (
                    z * 0.5 * (1.0 + erf(z / np.sqrt(2.0))))
    return out


# revision 14
# speedup vs baseline: 1.0264x; 1.0264x over previous
"""DCNv3 Trainium2 kernel: 8-core SPMD, core = (batch, group-pair).

Wire-minimal design for the axon tunnel (~45 MB/s H2D, ~28 MB/s D2H):
  - per-core inputs: xin [32, H*W] bf16 (the core's OWN 32 channels only),
    cstb/cstf consts. An in-kernel AllGather over pairs [[0,1],[2,3],...]
    reconstructs all 64 channels for the offset/mask conv on device.
  - on-device prologue rebuilds zero-padded grids (xfpo 64ch, xfpw 32ch)
    in DRAM.
  - per chunk: om = w_om @ x (PE); clamped-tri fields; A9'[(g,k),t] =
    sigmoid(ml)*ay_m*ax_n; A9' bounced through DRAM and broadcast 1->16
    partitions per (g,k) (k-major rows k*16+ch); x shifted copies DMA'd
    straight from xfp DRAM (plain slices); val = sum_t A9r * x_shift (DVE);
    conv via PE; GroupNorm + exact Gelu; outv in bf16.
  - dispatch: jit(shard_map(bass_exec)) built ONCE and cached; donated
    output buffers are created on-device by a tiny jitted zeros fn (no H2D).
    Mirrors bass_utils.run_bass_kernel_spmd's axon path (bass2jax); falls
    back to run_bass_kernel_spmd when not under axon.
Exact for |offset| <= 1; device outputs max|offset| and the host applies an
exact numpy correction for the (rare) larger offsets.
"""
import sys
import numpy as np
from contextlib import ExitStack

for _p in ("/opt/trn_rl_repo",):
    if _p not in sys.path:
        sys.path.insert(0, _p)

G, K, CG, H, W = 4, 9, 16, 128, 128
HP, WP = H + 4, W + 4
PX = HP * WP            # 17424
NPIX = H * W            # 16384
TS = WP + 1             # 133: tap-shift slack inside xr windows
MARG = 2 * WP + 8       # 272: margin so all shifted reads stay in-bounds
PXP = PX + 2 * MARG     # 17968
CHUNK = 484             # divides PX (36 chunks)
NCH = PX // CHUNK       # 36
CW = CHUNK + 2 * TS     # 750
EPS = 1e-5
NCORES = 8

_CACHE = {}


def _build_nc(mdt_name):
    import concourse.mybir as mybir
    from concourse import bacc, tile

    f32 = mybir.dt.float32
    mdt = getattr(mybir.dt, mdt_name)
    AF = mybir.ActivationFunctionType
    OP = mybir.AluOpType
    AX = mybir.AxisListType

    nc = bacc.Bacc("TRN2", target_bir_lowering=False, debug=False,
                   num_devices=NCORES)
    xin = nc.dram_tensor("xin", [32, NPIX], mdt, kind="ExternalInput")
    cstb = nc.dram_tensor("cstb", [128, 146], mdt, kind="ExternalInput")
    cstf = nc.dram_tensor("cstf", [36, 8], f32, kind="ExternalInput")
    outv = [nc.dram_tensor(f"outv{g}", [16, NPIX], mdt,
                           kind="ExternalOutput") for g in range(2)]
    statso = nc.dram_tensor("statso", [32, 2], f32, kind="ExternalOutput")
    moffo = nc.dram_tensor("moffo", [36, 1], f32, kind="ExternalOutput")

    with ExitStack() as ctx:
        tc = ctx.enter_context(tile.TileContext(nc))
        cpool = ctx.enter_context(tc.tile_pool(name="consts", bufs=1))
        ppool = ctx.enter_context(tc.tile_pool(name="psum", bufs=4, space="PSUM"))
        dpool = ctx.enter_context(tc.tile_pool(name="drsc", bufs=1, space="DRAM"))
        keep = ctx.enter_context(tc.tile_pool(name="keep", bufs=1))

        sb_cb = cpool.tile([128, 146], mdt)
        nc.sync.dma_start(sb_cb[:], cstb[:])
        sb_cf = cpool.tile([36, 8], f32)
        nc.sync.dma_start(sb_cf[:], cstf[:])
        sb_womT = sb_cb[0:64, 0:82]
        sb_wA = [sb_cb[:, 82:98], sb_cb[:, 98:114]]       # rows (k*16+ci)
        sb_wB = [sb_cb[0:16, 114:130], sb_cb[0:16, 130:146]]  # rows ci
        sb_bomYX = sb_cf[0:36, 0:1]
        sb_bomM = sb_cf[0:18, 1:2]
        sb_dcnb = [sb_cf[0:16, 2:3], sb_cf[0:16, 3:4]]
        sb_gnwf = [sb_cf[0:16, 4:5], sb_cf[0:16, 5:6]]
        sb_gnbf = [sb_cf[0:16, 6:7], sb_cf[0:16, 7:8]]

        vsb = [keep.tile([16, PX], mdt, tag=f"vsb{g}", name=f"vsb{g}")
               for g in range(2)]
        moffa = keep.tile([36, 1], f32)
        nc.vector.memset(moffa[:], 0.0)

        # ---- prologue: pair AllGather x, rebuild padded grids in DRAM ----
        # xfpo: gathered 64ch grid (om input, original channel order)
        # xfpw: own 32ch grid (shifted-window source)
        xfpo = dpool.tile([64, PXP], mdt, tag="xfpo", name="xfpo")
        xfpw = dpool.tile([32, PXP], mdt, tag="xfpw", name="xfpw")
        xb = dpool.tile([32, NPIX], mdt, tag="xb", name="xb")
        xg = dpool.tile([64, NPIX], mdt, tag="xg", name="xg")
        ZW = PXP // 4   # 4492
        with tc.tile_pool(name="pre", bufs=1) as pre:
            z64 = pre.tile([64, ZW], mdt)
            nc.vector.memset(z64[:], 0.0)
            for j in range(4):
                nc.sync.dma_start(xfpo[:, j * ZW:(j + 1) * ZW], z64[:])
                nc.sync.dma_start(xfpw[:, j * ZW:(j + 1) * ZW], z64[0:32, :])
            nc.gpsimd.dma_start(xb[:], xin[:])
            nc.gpsimd.collective_compute(
                "AllGather", mybir.AluOpType.bypass,
                replica_groups=[[0, 1], [2, 3], [4, 5], [6, 7]],
                ins=[xb[:].opt()], outs=[xg[:].opt()])
            xfpo_in = xfpo[:, MARG:MARG + PX].rearrange(
                "p (h w) -> p h w", w=WP)[:, 2:2 + H, 2:2 + W]
            nc.sync.dma_start(
                xfpo_in, xg[:].rearrange("p (h w) -> p h w", w=W))
            xfpw_in = xfpw[:, MARG:MARG + PX].rearrange(
                "p (h w) -> p h w", w=WP)[:, 2:2 + H, 2:2 + W]
            nc.sync.dma_start(
                xfpw_in, xin[:].rearrange("p (h w) -> p h w", w=W))

        # ----- fused per-chunk pipeline -----
        with tc.tile_pool(name="p2", bufs=2) as p2, \
             tc.tile_pool(name="pbig", bufs=1) as pbig, \
             tc.tile_pool(name="dscr", bufs=2, space="DRAM") as dscr:
            for c in range(NCH):
                q = c * CHUNK
                xc = p2.tile([64, CHUNK], mdt, tag="xc")
                nc.sync.dma_start(xc[:], xfpo[:, MARG + q:MARG + q + CHUNK])
                omYX = p2.tile([36, CHUNK], f32, tag="omYX")
                omM = p2.tile([18, CHUNK], f32, tag="omM")
                ps = ppool.tile([82, CHUNK], f32, tag="omps")
                nc.tensor.matmul(ps[:], sb_womT, xc[:], start=True, stop=True)
                nc.scalar.activation(omYX[:], ps[0:36, :],
                                     AF.Identity, bias=sb_bomYX)
                nc.scalar.activation(omM[:], ps[64:82, :],
                                     AF.Identity, bias=sb_bomM)
                ayx = p2.tile([36, 3, CHUNK], mdt, tag="ayx")
                for m in range(3):
                    tmp = p2.tile([36, CHUNK], f32, tag="tmp_m")
                    tabs = p2.tile([36, CHUNK], f32, tag="tabs_m")
                    nc.vector.tensor_scalar(tmp[:], omYX[:], float(1 - m),
                                            None, OP.add)
                    nc.vector.scalar_tensor_tensor(tabs[:], tmp[:], -1.0,
                                                   tmp[:], OP.mult, OP.max)
                    if m == 1:
                        mr = p2.tile([36, 1], f32, tag="mr")
                        nc.vector.tensor_reduce(mr[:], tabs[:], axis=AX.X,
                                                op=OP.max)
                        nc.vector.tensor_tensor(moffa[:], moffa[:], mr[:],
                                                OP.max)
                    nc.scalar.activation(ayx[:, m, :], tabs[:], AF.Relu,
                                         bias=1.0, scale=-1.0)
                ms = p2.tile([18, CHUNK], mdt, tag="ms")
                nc.scalar.activation(ms[:], omM[:], AF.Sigmoid)
                axT = p2.tile([18, 3, CHUNK], mdt, tag="axT")
                nc.sync.dma_start(axT[:], ayx[18:36, :, :])
                ayp = p2.tile([18, 3, CHUNK], mdt, tag="ayp")
                for m in range(3):
                    nc.vector.tensor_tensor(ayp[:, m, :], ayx[0:18, m, :],
                                            ms[:], OP.mult)
                a9p = p2.tile([18, K, CHUNK], mdt, tag="a9p")
                for t in range(K):
                    m, n = t // 3, t % 3
                    nc.vector.tensor_tensor(a9p[:, t, :], ayp[:, m, :],
                                            axT[:, n, :], OP.mult)
                # bounce A9' through DRAM, broadcast 1 row -> 16 partitions
                a9d = dscr.tile([18, K, CHUNK], mdt, tag="a9d")
                nc.sync.dma_start(a9d[:], a9p[:])
                a9rA = [pbig.tile([128, K, CHUNK], mdt, tag=f"a9rA{g}",
                                  name=f"a9rA{g}") for g in range(2)]
                a9rB = [pbig.tile([16, K, CHUNK], mdt, tag=f"a9rB{g}",
                                  name=f"a9rB{g}") for g in range(2)]
                for g in range(2):
                    for k in range(8):
                        nc.sync.dma_start(
                            a9rA[g][k * 16:(k + 1) * 16, :, :],
                            a9d[g * 9 + k:g * 9 + k + 1, :, :]
                            .to_broadcast((16, K, CHUNK)))
                    nc.sync.dma_start(
                        a9rB[g][:],
                        a9d[g * 9 + 8:g * 9 + 9, :, :]
                        .to_broadcast((16, K, CHUNK)))
                # shifted x windows straight from DRAM (k-major rows)
                xrA = [p2.tile([128, CW], mdt, tag=f"xrA{g}",
                               name=f"xrA{g}") for g in range(2)]
                xrB = [p2.tile([16, CW], mdt, tag=f"xrB{g}",
                               name=f"xrB{g}") for g in range(2)]
                for g in range(2):
                    r0 = 16 * g
                    for k in range(8):
                        soff = (k // 3 - 1) * WP + (k % 3 - 1)
                        c0 = MARG + q + soff - TS
                        nc.sync.dma_start(xrA[g][k * 16:(k + 1) * 16, :],
                                          xfpw[r0:r0 + 16, c0:c0 + CW])
                    c0 = MARG + q + (WP + 1) - TS
                    nc.sync.dma_start(xrB[g][:],
                                      xfpw[r0:r0 + 16, c0:c0 + CW])
                # modulation: val = sum_t A9r * x_shift
                val = [pbig.tile([128, CHUNK], mdt, tag=f"val{i}",
                                 name=f"val{i}") for i in range(2)]
                valB = [pbig.tile([16, CHUNK], mdt, tag=f"valB{g}",
                                  name=f"valB{g}") for g in range(2)]
                tiles = [(val[0], a9rA[0], xrA[0], "pA0"),
                         (val[1], a9rA[1], xrA[1], "pA1"),
                         (valB[0], a9rB[0], xrB[0], "pB0"),
                         (valB[1], a9rB[1], xrB[1], "pB1")]
                for vt, ar, xr, ptag in tiles:
                    rows = vt.shape[0]
                    prod = pbig.tile([rows, CHUNK], mdt, tag=ptag, name=ptag)
                    for t in range(K):
                        m, n = t // 3, t % 3
                        off = TS + (m - 1) * WP + (n - 1)
                        dst = vt if t == 0 else prod
                        nc.vector.tensor_tensor(dst[:], ar[:, t, :],
                                                xr[:, off:off + CHUNK],
                                                OP.mult)
                        if t > 0:
                            nc.vector.tensor_tensor(vt[:], vt[:], prod[:],
                                                    OP.add)
                # conv
                for g in range(2):
                    psv = ppool.tile([16, CHUNK], f32, tag="psv")
                    nc.tensor.matmul(psv[:], sb_wA[g], val[g][:],
                                     start=True, stop=False)
                    nc.tensor.matmul(psv[:], sb_wB[g], valB[g][:],
                                     start=False, stop=True)
                    nc.scalar.activation(vsb[g][:, q:q + CHUNK],
                                         psv[:], AF.Identity, bias=sb_dcnb[g])
        nc.sync.dma_start(moffo[:], moffa[:])

        # ---------------- GroupNorm + Gelu ----------------
        with tc.tile_pool(name="p3", bufs=1) as p3:
            VOFF = 2 * WP
            invN = 1.0 / (CG * NPIX)
            zero16 = p3.tile([1, 16], f32)
            nc.vector.memset(zero16[:], 0.0)
            for g in range(2):
                vg = vsb[g]
                vap = vg[:, VOFF:VOFF + H * WP].rearrange(
                    "p (h w) -> p h w", w=WP)[:, :, 2:2 + W]
                vsq = p3.tile([16, PX], f32, tag="vsq", name="vsq")
                nc.scalar.activation(vsq[:], vg[:], AF.Square)
                sqap = vsq[:, VOFF:VOFF + H * WP].rearrange(
                    "p (h w) -> p h w", w=WP)[:, :, 2:2 + W]
                r1 = p3.tile([16, H], f32, tag="r1", name="r1")
                s1 = p3.tile([16, 1], f32, tag="s1", name="s1")
                nc.vector.tensor_reduce(r1[:], vap, axis=AX.X, op=OP.add)
                nc.vector.tensor_reduce(s1[:], r1[:], axis=AX.X, op=OP.add)
                r2 = p3.tile([16, H], f32, tag="r2", name="r2")
                s2 = p3.tile([16, 1], f32, tag="s2", name="s2")
                nc.vector.tensor_reduce(r2[:], sqap, axis=AX.X, op=OP.add)
                nc.vector.tensor_reduce(s2[:], r2[:], axis=AX.X, op=OP.add)
                stats = p3.tile([16, 2], f32, tag="stats", name="stats")
                nc.vector.tensor_copy(stats[:, 0:1], s1[:])
                nc.vector.tensor_copy(stats[:, 1:2], s2[:])
                nc.sync.dma_start(statso[g * 16:g * 16 + 16, :], stats[:])
                scr1 = dpool.tile([16, 1], f32, tag="scr1", name="scr1")
                scr2 = dpool.tile([16, 1], f32, tag="scr2", name="scr2")
                nc.sync.dma_start(scr1[:], s1[:])
                nc.sync.dma_start(scr2[:], s2[:])
                s1t = p3.tile([1, 16], f32, tag="s1t", name="s1t")
                s2t = p3.tile([1, 16], f32, tag="s2t", name="s2t")
                nc.sync.dma_start(s1t[:], scr1[:].rearrange("p x -> x p"))
                nc.sync.dma_start(s2t[:], scr2[:].rearrange("p x -> x p"))
                mug = p3.tile([1, 1], f32, tag="mug", name="mug")
                e2g = p3.tile([1, 1], f32, tag="e2g", name="e2g")
                nc.vector.tensor_reduce(mug[:], s1t[:], axis=AX.X, op=OP.add)
                nc.vector.tensor_reduce(e2g[:], s2t[:], axis=AX.X, op=OP.add)
                nc.vector.tensor_scalar(mug[:], mug[:], invN, None, OP.mult)
                nc.vector.tensor_scalar(e2g[:], e2g[:], invN, None, OP.mult)
                var = p3.tile([1, 1], f32, tag="var", name="var")
                nc.vector.tensor_tensor(var[:], mug[:], mug[:], OP.mult)
                nc.vector.tensor_tensor(var[:], e2g[:], var[:], OP.subtract)
                nc.vector.tensor_scalar(var[:], var[:], EPS, None, OP.add)
                sd = p3.tile([1, 1], f32, tag="sd", name="sd")
                nc.scalar.activation(sd[:], var[:], AF.Sqrt)
                ivg = p3.tile([1, 1], f32, tag="ivg", name="ivg")
                nc.vector.reciprocal(ivg[:], sd[:])
                inv16 = p3.tile([1, 16], f32, tag="inv16", name="inv16")
                mu16 = p3.tile([1, 16], f32, tag="mu16", name="mu16")
                nc.scalar.activation(inv16[:], zero16[:], AF.Identity,
                                     bias=ivg[:])
                nc.scalar.activation(mu16[:], zero16[:], AF.Identity,
                                     bias=mug[:])
                ivp = p3.tile([16, 1], f32, tag="ivp", name="ivp")
                mup = p3.tile([16, 1], f32, tag="mup", name="mup")
                scr3 = dpool.tile([1, 16], f32, tag="scr3", name="scr3")
                scr4 = dpool.tile([1, 16], f32, tag="scr4", name="scr4")
                nc.sync.dma_start(scr3[:], inv16[:])
                nc.sync.dma_start(scr4[:], mu16[:])
                nc.sync.dma_start(ivp[:], scr3[:].rearrange("x p -> p x"))
                nc.sync.dma_start(mup[:], scr4[:].rearrange("x p -> p x"))
                scp = p3.tile([16, 1], f32, tag="scp", name="scp")
                bip = p3.tile([16, 1], f32, tag="bip", name="bip")
                nc.vector.tensor_tensor(scp[:], sb_gnwf[g], ivp[:], OP.mult)
                nc.vector.tensor_tensor(bip[:], mup[:], scp[:], OP.mult)
                nc.vector.tensor_tensor(bip[:], sb_gnbf[g], bip[:],
                                        OP.subtract)
                og = p3.tile([16, PX], mdt, tag="og", name="og")
                nc.scalar.activation(og[:], vg[:], AF.Gelu,
                                     bias=bip[:], scale=scp[:])
                ogap = og[:, VOFF:VOFF + H * WP].rearrange(
                    "p (h w) -> p h w", w=WP)[:, :, 2:2 + W]
                nc.sync.dma_start(
                    outv[g][:, :].rearrange("p (h w) -> p h w", w=W),
                    ogap)

    if not nc.is_finalized():
        nc.finalize()
    return nc


def get_nc(mdt_name="bfloat16"):
    key = ("nc", mdt_name)
    if key not in _CACHE:
        _CACHE[key] = _build_nc(mdt_name)
    return _CACHE[key]


def _host_prep(x, w_om, b_om, dcn_w, dcn_b, gn_w, gn_b, offset_scale, cast):
    B = x.shape[0]
    sc = float(np.asarray(offset_scale).reshape(-1)[0])
    idx_oy = [g * 27 + 2 * k for g in range(G) for k in range(K)]
    idx_ox = [g * 27 + 2 * k + 1 for g in range(G) for k in range(K)]
    idx_ml = [g * 27 + 18 + k for g in range(G) for k in range(K)]
    xb = np.ascontiguousarray(x.reshape(B, 64, NPIX)).astype(cast)
    in_maps = []
    for core in range(NCORES):
        b, gp = core // 2, core % 2
        gsel = [2 * gp, 2 * gp + 1]
        own = slice(gsel[0] * CG, gsel[0] * CG + 2 * CG)
        m = {"xin": xb[b][own]}
        cols = []
        for idx in (idx_oy, idx_ox, idx_ml):
            for g in gsel:
                cols += idx[g * K:(g + 1) * K]
        wsel = w_om[cols].astype(np.float32).copy()
        bsel = b_om[cols].astype(np.float32).copy()
        wsel[:36] *= sc
        bsel[:36] *= sc
        cb = np.zeros((128, 146), np.float32)
        cb[0:64, 0:82] = np.concatenate(
            [wsel[0:36], np.zeros((28, 64), np.float32), wsel[36:54]]).T
        for gi in range(2):
            wg = dcn_w[gsel[gi]].reshape(CG, CG, K)   # [co, ci, k]
            # k-major rows (k*16+ci), cols co
            cb[:, 82 + 16 * gi:98 + 16 * gi] = np.transpose(
                wg[:, :, :8], (2, 1, 0)).reshape(128, CG)
            cb[0:16, 114 + 16 * gi:130 + 16 * gi] = wg[:, :, 8].T
        m["cstb"] = cb.astype(cast)
        cf = np.zeros((36, 8), np.float32)
        cf[0:36, 0] = bsel[0:36]
        cf[0:18, 1] = bsel[36:54]
        cf[0:16, 2] = dcn_b[gsel[0]]
        cf[0:16, 3] = dcn_b[gsel[1]]
        c0 = gsel[0] * CG
        cf[0:16, 4] = gn_w[c0:c0 + 16]
        cf[0:16, 5] = gn_w[c0 + 16:c0 + 32]
        cf[0:16, 6] = gn_b[c0:c0 + 16]
        cf[0:16, 7] = gn_b[c0 + 16:c0 + 32]
        m["cstf"] = cf
        in_maps.append(m)
    return in_maps


# ---------------- cached PJRT dispatch (axon path) ----------------

def _get_exec(mdt_name="bfloat16"):
    key = ("exec", mdt_name)
    if key in _CACHE:
        return _CACHE[key]
    import jax
    import jax.numpy as jnp
    from jax.sharding import Mesh, PartitionSpec, NamedSharding
    from jax.experimental.shard_map import shard_map
    from concourse import bass2jax
    import concourse.mybir as mybir

    nc = get_nc(mdt_name)
    bass2jax.install_neuronx_cc_hook()
    partition_name = (nc.partition_id_tensor.name
                      if nc.partition_id_tensor else None)
    in_names, out_names, out_avals, out_np = [], [], [], []
    for alloc in nc.m.functions[0].allocations:
        if not isinstance(alloc, mybir.MemoryLocationSet):
            continue
        name = alloc.memorylocations[0].name
        if alloc.kind == "ExternalInput":
            if name != partition_name:
                in_names.append(name)
        elif alloc.kind == "ExternalOutput":
            shape = tuple(alloc.tensor_shape)
            dtype = mybir.dt.np(alloc.dtype)
            out_names.append(name)
            out_avals.append(jax.core.ShapedArray(shape, dtype))
            out_np.append((shape, dtype))
    n_params = len(in_names)
    n_outs = len(out_names)
    in_names_all = list(in_names) + list(out_names)
    if partition_name is not None:
        in_names_all.append(partition_name)
    donate = tuple(range(n_params, n_params + n_outs))

    def _body(*args):
        operands = list(args)
        if partition_name is not None:
            operands.append(bass2jax.partition_id_tensor())
        outs = bass2jax._bass_exec_p.bind(
            *operands,
            out_avals=tuple(out_avals),
            in_names=tuple(in_names_all),
            out_names=tuple(out_names),
            lowering_input_output_aliases=(),
            sim_require_finite=True,
            sim_require_nnan=True,
            nc=nc,
        )
        return tuple(outs)

    devices = jax.devices()[:NCORES]
    mesh = Mesh(np.asarray(devices), ("core",))
    pspec = PartitionSpec("core")
    in_specs = (pspec,) * (n_params + n_outs)
    out_specs = (pspec,) * n_outs
    sharded = jax.jit(
        shard_map(_body, mesh=mesh, in_specs=in_specs, out_specs=out_specs,
                  check_rep=False),
        donate_argnums=donate, keep_unused=True)
    sh = NamedSharding(mesh, pspec)

    zdtypes = []
    import ml_dtypes
    for s, d in out_np:
        zdtypes.append(jnp.bfloat16 if d == ml_dtypes.bfloat16 else d)

    def _mk_zeros():
        return tuple(
            jnp.zeros((NCORES * s[0],) + tuple(s[1:]), zd)
            for (s, d), zd in zip(out_np, zdtypes))

    zeros_fn = jax.jit(_mk_zeros, out_shardings=(sh,) * n_outs)
    E = dict(sharded=sharded, zeros_fn=zeros_fn, in_names=in_names,
             out_names=out_names, out_np=out_np)
    _CACHE[key] = E
    return E


def _dispatch(in_maps, mdt_name="bfloat16"):
    """concat per-core maps, run on 8 cores, return per-core result dicts.

    Includes host concat + on-device zero-output creation + execute + fetch:
    the same work run_bass_kernel_spmd would do per call, with the jit cached.
    """
    E = _get_exec(mdt_name)
    concat_in = [np.concatenate([m[name] for m in in_maps], axis=0)
                 for name in E["in_names"]]
    zeros = _CACHE.pop(("zeros_next", mdt_name), None) or E["zeros_fn"]()
    out_arrs = E["sharded"](*concat_in, *zeros)
    # prefetch donated buffers for the NEXT call while this one runs
    _CACHE[("zeros_next", mdt_name)] = E["zeros_fn"]()
    for a in out_arrs:
        for s_ in a.addressable_shards:
            s_.data.copy_to_host_async()
    outs = [np.asarray(a) for a in out_arrs]
    results = []
    for c in range(NCORES):
        r = {}
        for i, name in enumerate(E["out_names"]):
            s0 = E["out_np"][i][0][0]
            r[name] = outs[i][c * s0:(c + 1) * s0]
        results.append(r)
    return results


def _run_spmd(nc, in_maps):
    """Fallback: stock dispatcher (non-axon environments)."""
    from concourse.bass_utils import run_bass_kernel_spmd
    res = run_bass_kernel_spmd(nc, in_maps, core_ids=list(range(NCORES)))
    return res.results


def kernel(x, w_om, b_om, dcn_w, dcn_b, gn_w, gn_b, offset_scale,
           _mdt="bfloat16"):
    import ml_dtypes

    x = np.asarray(x, np.float32)
    w_om = np.asarray(w_om, np.float32)
    b_om = np.asarray(b_om, np.float32)
    dcn_w = np.asarray(dcn_w, np.float32)
    dcn_b = np.asarray(dcn_b, np.float32)
    gn_w = np.asarray(gn_w, np.float32)
    gn_b = np.asarray(gn_b, np.float32)
    offset_scale = np.asarray(offset_scale, np.float32)
    cast = ml_dtypes.bfloat16 if _mdt == "bfloat16" else np.float32
    in_maps = _host_prep(x, w_om, b_om, dcn_w, dcn_b, gn_w, gn_b,
                         offset_scale, cast)
    try:
        from concourse.bass_utils import axon_active
        use_fast = axon_active()
    except Exception:
        use_fast = False
    if use_fast:
        results = _dispatch(in_maps, _mdt)
    else:
        results = _run_spmd(get_nc(_mdt), in_maps)
    out = np.zeros((4, 64, H, W), np.float32)
    stats = np.zeros((8, 32, 2), np.float32)
    moff_all = 0.0
    for core in range(NCORES):
        b, gp = core // 2, core % 2
        r = results[core]
        out[b, gp * 32:gp * 32 + 16] = np.asarray(
            r["outv0"], np.float32).reshape(16, H, W)
        out[b, gp * 32 + 16:gp * 32 + 32] = np.asarray(
            r["outv1"], np.float32).reshape(16, H, W)
        stats[core] = r["statso"]
        moff_all = max(moff_all, float(np.max(r["moffo"])))
    if moff_all > 1.0:
        out = _host_correct(out, stats, x, w_om, b_om, dcn_w, dcn_b,
                            gn_w, gn_b, offset_scale)
    return out


def _host_correct(out, stats, x, w_om, b_om, dcn_w, dcn_b, gn_w, gn_b,
                  offset_scale):
    """Exact fix for rare pixels with |offset| > 1 (clamped-tri mismatch)."""
    from scipy.special import erf, expit
    sc = float(np.asarray(offset_scale).reshape(-1)[0])
    B = x.shape[0]
    om = (np.einsum('bcp,oc->bop', x.reshape(B, 64, NPIX), w_om)
          + b_om[None, :, None]).reshape(B, 108, H, W)
    for b in range(B):
        for g in range(G):
            oy = om[b, g * 27:g * 27 + 18:2] * sc
            ox = om[b, g * 27 + 1:g * 27 + 18:2] * sc
            bad = (np.abs(oy) > 1).any(0) | (np.abs(ox) > 1).any(0)
            if not bad.any():
                continue
            ml = expit(om[b, g * 27 + 18:g * 27 + 27])
            core = b * 2 + g // 2
            gl = (g % 2) * 16
            N = CG * NPIX
            mu = stats[core, gl:gl + 16, 0].sum() / N
            var = stats[core, gl:gl + 16, 1].sum() / N - mu * mu
            inv = 1.0 / np.sqrt(var + EPS)
            wg = dcn_w[g].reshape(CG, CG, K)
            for hh, ww in zip(*np.nonzero(bad)):
                val = np.zeros((CG, K), np.float32)
                for k in range(K):
                    ky, kx = k // 3, k % 3
                    py = hh + ky - 1 + oy[k, hh, ww]
                    pxx = ww + kx - 1 + ox[k, hh, ww]
                    y0, x0 = int(np.floor(py)), int(np.floor(pxx))
                    fy, fx = py - y0, pxx - x0
                    acc = np.zeros(CG, np.float32)
                    for dy, wy in ((0, 1 - fy), (1, fy)):
                        for dx, wx in ((0, 1 - fx), (1, fx)):
                            yy, xx = y0 + dy, x0 + dx
                            if 0 <= yy < H and 0 <= xx < W:
                                acc += wy * wx * x[b, g * CG:g * CG + CG,
                                                   yy, xx]
                    val[:, k] = acc * ml[k, hh, ww]
                pre = np.einsum('ck,ock->o', val, wg) + dcn_b[g]
                z = ((pre - mu) * inv * gn_w[g * CG:g * CG + CG]
                     + gn_b[g * CG:g * CG + CG])
                out[b, g * CG:g * CG + CG, hh, ww] = (
                    z * 0.5 * (1.0 + erf(z / np.sqrt(2.0))))
    return out


# revision 15
# speedup vs baseline: 1.0851x; 1.0572x over previous
"""DCNv3 Trainium2 kernel: 8-core SPMD, core = (batch, group-pair).

Wire-minimal design for the axon tunnel (~45 MB/s H2D, ~28 MB/s D2H):
  - per-core inputs: xin [32, H*W] bf16 (the core's OWN 32 channels only),
    cstb/cstf consts. An in-kernel AllGather over pairs [[0,1],[2,3],...]
    reconstructs all 64 channels for the offset/mask conv on device.
  - on-device prologue rebuilds zero-padded grids (xfpo 64ch, xfpw 32ch)
    in DRAM.
  - per chunk: om = w_om @ x (PE); clamped-tri fields; A9'[(g,k),t] =
    sigmoid(ml)*ay_m*ax_n; A9' bounced through DRAM and broadcast 1->16
    partitions per (g,k) (k-major rows k*16+ch); x shifted copies DMA'd
    straight from xfp DRAM (plain slices); val = sum_t A9r * x_shift (DVE);
    conv via PE; GroupNorm + exact Gelu; outv in bf16.
  - dispatch: jit(shard_map(bass_exec)) built ONCE and cached; donated
    output buffers are created on-device by a tiny jitted zeros fn (no H2D).
    Mirrors bass_utils.run_bass_kernel_spmd's axon path (bass2jax); falls
    back to run_bass_kernel_spmd when not under axon.
Exact for |offset| <= 1; device outputs max|offset| and the host applies an
exact numpy correction for the (rare) larger offsets.
"""
import sys
import numpy as np
from contextlib import ExitStack

for _p in ("/opt/trn_rl_repo",):
    if _p not in sys.path:
        sys.path.insert(0, _p)

G, K, CG, H, W = 4, 9, 16, 128, 128
HP, WP = H + 4, W + 4
PX = HP * WP            # 17424
NPIX = H * W            # 16384
TS = WP + 1             # 133: tap-shift slack inside xr windows
MARG = 2 * WP + 8       # 272: margin so all shifted reads stay in-bounds
PXP = PX + 2 * MARG     # 17968
CHUNK = 484             # divides PX (36 chunks)
NCH = PX // CHUNK       # 36
CW = CHUNK + 2 * TS     # 750
EPS = 1e-5
NCORES = 8

_CACHE = {}


def _build_nc(mdt_name):
    import concourse.mybir as mybir
    from concourse import bacc, tile

    f32 = mybir.dt.float32
    mdt = getattr(mybir.dt, mdt_name)
    AF = mybir.ActivationFunctionType
    OP = mybir.AluOpType
    AX = mybir.AxisListType

    nc = bacc.Bacc("TRN2", target_bir_lowering=False, debug=False,
                   num_devices=NCORES)
    xin = nc.dram_tensor("xin", [32, NPIX], mdt, kind="ExternalInput")
    cstb = nc.dram_tensor("cstb", [128, 146], mdt, kind="ExternalInput")
    cstf = nc.dram_tensor("cstf", [36, 8], f32, kind="ExternalInput")
    outv = [nc.dram_tensor(f"outv{g}", [16, NPIX], mdt,
                           kind="ExternalOutput") for g in range(2)]
    statso = nc.dram_tensor("statso", [32, 2], f32, kind="ExternalOutput")
    moffo = nc.dram_tensor("moffo", [36, 1], f32, kind="ExternalOutput")

    with ExitStack() as ctx:
        tc = ctx.enter_context(tile.TileContext(nc))
        cpool = ctx.enter_context(tc.tile_pool(name="consts", bufs=1))
        ppool = ctx.enter_context(tc.tile_pool(name="psum", bufs=4, space="PSUM"))
        dpool = ctx.enter_context(tc.tile_pool(name="drsc", bufs=1, space="DRAM"))
        keep = ctx.enter_context(tc.tile_pool(name="keep", bufs=1))

        sb_cb = cpool.tile([128, 146], mdt)
        nc.sync.dma_start(sb_cb[:], cstb[:])
        sb_cf = cpool.tile([36, 8], f32)
        nc.sync.dma_start(sb_cf[:], cstf[:])
        sb_womT = sb_cb[0:64, 0:82]
        sb_wA = [sb_cb[:, 82:98], sb_cb[:, 98:114]]       # rows (k*16+ci)
        sb_wB = [sb_cb[0:16, 114:130], sb_cb[0:16, 130:146]]  # rows ci
        sb_bomYX = sb_cf[0:36, 0:1]
        sb_bomM = sb_cf[0:18, 1:2]
        sb_dcnb = [sb_cf[0:16, 2:3], sb_cf[0:16, 3:4]]
        sb_gnwf = [sb_cf[0:16, 4:5], sb_cf[0:16, 5:6]]
        sb_gnbf = [sb_cf[0:16, 6:7], sb_cf[0:16, 7:8]]

        vsb = [keep.tile([16, PX], mdt, tag=f"vsb{g}", name=f"vsb{g}")
               for g in range(2)]
        moffa = keep.tile([36, 1], f32)
        nc.vector.memset(moffa[:], 0.0)

        # ---- prologue: pair AllGather x, rebuild padded grids in DRAM ----
        # xfpo: gathered 64ch grid (om input, original channel order)
        # xfpw: own 32ch grid (shifted-window source)
        xfpo = dpool.tile([64, PXP], mdt, tag="xfpo", name="xfpo")
        xfpw = dpool.tile([32, PXP], mdt, tag="xfpw", name="xfpw")
        xb = dpool.tile([32, NPIX], mdt, tag="xb", name="xb")
        xg = dpool.tile([64, NPIX], mdt, tag="xg", name="xg")
        ZW = PXP // 4   # 4492
        with tc.tile_pool(name="pre", bufs=1) as pre:
            z64 = pre.tile([64, ZW], mdt)
            nc.vector.memset(z64[:], 0.0)
            for j in range(4):
                nc.sync.dma_start(xfpo[:, j * ZW:(j + 1) * ZW], z64[:])
                nc.sync.dma_start(xfpw[:, j * ZW:(j + 1) * ZW], z64[0:32, :])
            nc.gpsimd.dma_start(xb[:], xin[:])
            nc.gpsimd.collective_compute(
                "AllGather", mybir.AluOpType.bypass,
                replica_groups=[[0, 1], [2, 3], [4, 5], [6, 7]],
                ins=[xb[:].opt()], outs=[xg[:].opt()])
            xfpo_in = xfpo[:, MARG:MARG + PX].rearrange(
                "p (h w) -> p h w", w=WP)[:, 2:2 + H, 2:2 + W]
            nc.sync.dma_start(
                xfpo_in, xg[:].rearrange("p (h w) -> p h w", w=W))
            xfpw_in = xfpw[:, MARG:MARG + PX].rearrange(
                "p (h w) -> p h w", w=WP)[:, 2:2 + H, 2:2 + W]
            nc.sync.dma_start(
                xfpw_in, xin[:].rearrange("p (h w) -> p h w", w=W))

        # ----- fused per-chunk pipeline -----
        with tc.tile_pool(name="p2", bufs=2) as p2, \
             tc.tile_pool(name="pbig", bufs=1) as pbig, \
             tc.tile_pool(name="dscr", bufs=2, space="DRAM") as dscr:
            for c in range(NCH):
                q = c * CHUNK
                xc = p2.tile([64, CHUNK], mdt, tag="xc")
                nc.sync.dma_start(xc[:], xfpo[:, MARG + q:MARG + q + CHUNK])
                omYX = p2.tile([36, CHUNK], f32, tag="omYX")
                omM = p2.tile([18, CHUNK], f32, tag="omM")
                ps = ppool.tile([82, CHUNK], f32, tag="omps")
                nc.tensor.matmul(ps[:], sb_womT, xc[:], start=True, stop=True)
                nc.scalar.activation(omYX[:], ps[0:36, :],
                                     AF.Identity, bias=sb_bomYX)
                nc.scalar.activation(omM[:], ps[64:82, :],
                                     AF.Identity, bias=sb_bomM)
                ayx = p2.tile([36, 3, CHUNK], mdt, tag="ayx")
                for m in range(3):
                    tmp = p2.tile([36, CHUNK], f32, tag="tmp_m")
                    tabs = p2.tile([36, CHUNK], f32, tag="tabs_m")
                    nc.vector.tensor_scalar(tmp[:], omYX[:], float(1 - m),
                                            None, OP.add)
                    nc.vector.scalar_tensor_tensor(tabs[:], tmp[:], -1.0,
                                                   tmp[:], OP.mult, OP.max)
                    if m == 1:
                        mr = p2.tile([36, 1], f32, tag="mr")
                        nc.vector.tensor_reduce(mr[:], tabs[:], axis=AX.X,
                                                op=OP.max)
                        nc.vector.tensor_tensor(moffa[:], moffa[:], mr[:],
                                                OP.max)
                    nc.scalar.activation(ayx[:, m, :], tabs[:], AF.Relu,
                                         bias=1.0, scale=-1.0)
                ms = p2.tile([18, CHUNK], mdt, tag="ms")
                nc.scalar.activation(ms[:], omM[:], AF.Sigmoid)
                axT = p2.tile([18, 3, CHUNK], mdt, tag="axT")
                nc.sync.dma_start(axT[:], ayx[18:36, :, :])
                ayp = p2.tile([18, 3, CHUNK], mdt, tag="ayp")
                for m in range(3):
                    nc.vector.tensor_tensor(ayp[:, m, :], ayx[0:18, m, :],
                                            ms[:], OP.mult)
                a9p = p2.tile([18, K, CHUNK], mdt, tag="a9p")
                for t in range(K):
                    m, n = t // 3, t % 3
                    nc.vector.tensor_tensor(a9p[:, t, :], ayp[:, m, :],
                                            axT[:, n, :], OP.mult)
                # bounce A9' through DRAM, broadcast 1 row -> 16 partitions
                a9d = dscr.tile([18, K, CHUNK], mdt, tag="a9d")
                nc.sync.dma_start(a9d[:], a9p[:])
                a9rA = [pbig.tile([128, K, CHUNK], mdt, tag=f"a9rA{g}",
                                  name=f"a9rA{g}") for g in range(2)]
                a9rB = [pbig.tile([16, K, CHUNK], mdt, tag=f"a9rB{g}",
                                  name=f"a9rB{g}") for g in range(2)]
                for g in range(2):
                    for k in range(8):
                        nc.sync.dma_start(
                            a9rA[g][k * 16:(k + 1) * 16, :, :],
                            a9d[g * 9 + k:g * 9 + k + 1, :, :]
                            .to_broadcast((16, K, CHUNK)))
                    nc.sync.dma_start(
                        a9rB[g][:],
                        a9d[g * 9 + 8:g * 9 + 9, :, :]
                        .to_broadcast((16, K, CHUNK)))
                # shifted x windows straight from DRAM (k-major rows)
                xrA = [p2.tile([128, CW], mdt, tag=f"xrA{g}",
                               name=f"xrA{g}") for g in range(2)]
                xrB = [p2.tile([16, CW], mdt, tag=f"xrB{g}",
                               name=f"xrB{g}") for g in range(2)]
                for g in range(2):
                    r0 = 16 * g
                    for k in range(8):
                        soff = (k // 3 - 1) * WP + (k % 3 - 1)
                        c0 = MARG + q + soff - TS
                        nc.sync.dma_start(xrA[g][k * 16:(k + 1) * 16, :],
                                          xfpw[r0:r0 + 16, c0:c0 + CW])
                    c0 = MARG + q + (WP + 1) - TS
                    nc.sync.dma_start(xrB[g][:],
                                      xfpw[r0:r0 + 16, c0:c0 + CW])
                # modulation: val = sum_t A9r * x_shift
                val = [pbig.tile([128, CHUNK], mdt, tag=f"val{i}",
                                 name=f"val{i}") for i in range(2)]
                valB = [pbig.tile([16, CHUNK], mdt, tag=f"valB{g}",
                                  name=f"valB{g}") for g in range(2)]
                tiles = [(val[0], a9rA[0], xrA[0], "pA0"),
                         (val[1], a9rA[1], xrA[1], "pA1"),
                         (valB[0], a9rB[0], xrB[0], "pB0"),
                         (valB[1], a9rB[1], xrB[1], "pB1")]
                for vt, ar, xr, ptag in tiles:
                    rows = vt.shape[0]
                    prod = pbig.tile([rows, CHUNK], mdt, tag=ptag, name=ptag)
                    for t in range(K):
                        m, n = t // 3, t % 3
                        off = TS + (m - 1) * WP + (n - 1)
                        dst = vt if t == 0 else prod
                        nc.vector.tensor_tensor(dst[:], ar[:, t, :],
                                                xr[:, off:off + CHUNK],
                                                OP.mult)
                        if t > 0:
                            nc.vector.tensor_tensor(vt[:], vt[:], prod[:],
                                                    OP.add)
                # conv
                for g in range(2):
                    psv = ppool.tile([16, CHUNK], f32, tag="psv")
                    nc.tensor.matmul(psv[:], sb_wA[g], val[g][:],
                                     start=True, stop=False)
                    nc.tensor.matmul(psv[:], sb_wB[g], valB[g][:],
                                     start=False, stop=True)
                    nc.scalar.activation(vsb[g][:, q:q + CHUNK],
                                         psv[:], AF.Identity, bias=sb_dcnb[g])
        nc.sync.dma_start(moffo[:], moffa[:])

        # ---------------- GroupNorm + Gelu ----------------
        with tc.tile_pool(name="p3", bufs=1) as p3:
            VOFF = 2 * WP
            invN = 1.0 / (CG * NPIX)
            zero16 = p3.tile([1, 16], f32)
            nc.vector.memset(zero16[:], 0.0)
            for g in range(2):
                vg = vsb[g]
                vap = vg[:, VOFF:VOFF + H * WP].rearrange(
                    "p (h w) -> p h w", w=WP)[:, :, 2:2 + W]
                vsq = p3.tile([16, PX], f32, tag="vsq", name="vsq")
                nc.scalar.activation(vsq[:], vg[:], AF.Square)
                sqap = vsq[:, VOFF:VOFF + H * WP].rearrange(
                    "p (h w) -> p h w", w=WP)[:, :, 2:2 + W]
                r1 = p3.tile([16, H], f32, tag="r1", name="r1")
                s1 = p3.tile([16, 1], f32, tag="s1", name="s1")
                nc.vector.tensor_reduce(r1[:], vap, axis=AX.X, op=OP.add)
                nc.vector.tensor_reduce(s1[:], r1[:], axis=AX.X, op=OP.add)
                r2 = p3.tile([16, H], f32, tag="r2", name="r2")
                s2 = p3.tile([16, 1], f32, tag="s2", name="s2")
                nc.vector.tensor_reduce(r2[:], sqap, axis=AX.X, op=OP.add)
                nc.vector.tensor_reduce(s2[:], r2[:], axis=AX.X, op=OP.add)
                stats = p3.tile([16, 2], f32, tag="stats", name="stats")
                nc.vector.tensor_copy(stats[:, 0:1], s1[:])
                nc.vector.tensor_copy(stats[:, 1:2], s2[:])
                nc.sync.dma_start(statso[g * 16:g * 16 + 16, :], stats[:])
                scr1 = dpool.tile([16, 1], f32, tag="scr1", name="scr1")
                scr2 = dpool.tile([16, 1], f32, tag="scr2", name="scr2")
                nc.sync.dma_start(scr1[:], s1[:])
                nc.sync.dma_start(scr2[:], s2[:])
                s1t = p3.tile([1, 16], f32, tag="s1t", name="s1t")
                s2t = p3.tile([1, 16], f32, tag="s2t", name="s2t")
                nc.sync.dma_start(s1t[:], scr1[:].rearrange("p x -> x p"))
                nc.sync.dma_start(s2t[:], scr2[:].rearrange("p x -> x p"))
                mug = p3.tile([1, 1], f32, tag="mug", name="mug")
                e2g = p3.tile([1, 1], f32, tag="e2g", name="e2g")
                nc.vector.tensor_reduce(mug[:], s1t[:], axis=AX.X, op=OP.add)
                nc.vector.tensor_reduce(e2g[:], s2t[:], axis=AX.X, op=OP.add)
                nc.vector.tensor_scalar(mug[:], mug[:], invN, None, OP.mult)
                nc.vector.tensor_scalar(e2g[:], e2g[:], invN, None, OP.mult)
                var = p3.tile([1, 1], f32, tag="var", name="var")
                nc.vector.tensor_tensor(var[:], mug[:], mug[:], OP.mult)
                nc.vector.tensor_tensor(var[:], e2g[:], var[:], OP.subtract)
                nc.vector.tensor_scalar(var[:], var[:], EPS, None, OP.add)
                sd = p3.tile([1, 1], f32, tag="sd", name="sd")
                nc.scalar.activation(sd[:], var[:], AF.Sqrt)
                ivg = p3.tile([1, 1], f32, tag="ivg", name="ivg")
                nc.vector.reciprocal(ivg[:], sd[:])
                inv16 = p3.tile([1, 16], f32, tag="inv16", name="inv16")
                mu16 = p3.tile([1, 16], f32, tag="mu16", name="mu16")
                nc.scalar.activation(inv16[:], zero16[:], AF.Identity,
                                     bias=ivg[:])
                nc.scalar.activation(mu16[:], zero16[:], AF.Identity,
                                     bias=mug[:])
                ivp = p3.tile([16, 1], f32, tag="ivp", name="ivp")
                mup = p3.tile([16, 1], f32, tag="mup", name="mup")
                scr3 = dpool.tile([1, 16], f32, tag="scr3", name="scr3")
                scr4 = dpool.tile([1, 16], f32, tag="scr4", name="scr4")
                nc.sync.dma_start(scr3[:], inv16[:])
                nc.sync.dma_start(scr4[:], mu16[:])
                nc.sync.dma_start(ivp[:], scr3[:].rearrange("x p -> p x"))
                nc.sync.dma_start(mup[:], scr4[:].rearrange("x p -> p x"))
                scp = p3.tile([16, 1], f32, tag="scp", name="scp")
                bip = p3.tile([16, 1], f32, tag="bip", name="bip")
                nc.vector.tensor_tensor(scp[:], sb_gnwf[g], ivp[:], OP.mult)
                nc.vector.tensor_tensor(bip[:], mup[:], scp[:], OP.mult)
                nc.vector.tensor_tensor(bip[:], sb_gnbf[g], bip[:],
                                        OP.subtract)
                og = p3.tile([16, PX], mdt, tag="og", name="og")
                nc.scalar.activation(og[:], vg[:], AF.Gelu,
                                     bias=bip[:], scale=scp[:])
                ogap = og[:, VOFF:VOFF + H * WP].rearrange(
                    "p (h w) -> p h w", w=WP)[:, :, 2:2 + W]
                nc.sync.dma_start(
                    outv[g][:, :].rearrange("p (h w) -> p h w", w=W),
                    ogap)

    if not nc.is_finalized():
        nc.finalize()
    return nc


def get_nc(mdt_name="bfloat16"):
    key = ("nc", mdt_name)
    if key not in _CACHE:
        _CACHE[key] = _build_nc(mdt_name)
    return _CACHE[key]


def _host_prep(x, w_om, b_om, dcn_w, dcn_b, gn_w, gn_b, offset_scale, cast):
    B = x.shape[0]
    sc = float(np.asarray(offset_scale).reshape(-1)[0])
    idx_oy = [g * 27 + 2 * k for g in range(G) for k in range(K)]
    idx_ox = [g * 27 + 2 * k + 1 for g in range(G) for k in range(K)]
    idx_ml = [g * 27 + 18 + k for g in range(G) for k in range(K)]
    xb = np.ascontiguousarray(x.reshape(B, 64, NPIX)).astype(cast)
    in_maps = []
    for core in range(NCORES):
        b, gp = core // 2, core % 2
        gsel = [2 * gp, 2 * gp + 1]
        own = slice(gsel[0] * CG, gsel[0] * CG + 2 * CG)
        m = {"xin": xb[b][own]}
        cols = []
        for idx in (idx_oy, idx_ox, idx_ml):
            for g in gsel:
                cols += idx[g * K:(g + 1) * K]
        wsel = w_om[cols].astype(np.float32).copy()
        bsel = b_om[cols].astype(np.float32).copy()
        wsel[:36] *= sc
        bsel[:36] *= sc
        cb = np.zeros((128, 146), np.float32)
        cb[0:64, 0:82] = np.concatenate(
            [wsel[0:36], np.zeros((28, 64), np.float32), wsel[36:54]]).T
        for gi in range(2):
            wg = dcn_w[gsel[gi]].reshape(CG, CG, K)   # [co, ci, k]
            # k-major rows (k*16+ci), cols co
            cb[:, 82 + 16 * gi:98 + 16 * gi] = np.transpose(
                wg[:, :, :8], (2, 1, 0)).reshape(128, CG)
            cb[0:16, 114 + 16 * gi:130 + 16 * gi] = wg[:, :, 8].T
        m["cstb"] = cb.astype(cast)
        cf = np.zeros((36, 8), np.float32)
        cf[0:36, 0] = bsel[0:36]
        cf[0:18, 1] = bsel[36:54]
        cf[0:16, 2] = dcn_b[gsel[0]]
        cf[0:16, 3] = dcn_b[gsel[1]]
        c0 = gsel[0] * CG
        cf[0:16, 4] = gn_w[c0:c0 + 16]
        cf[0:16, 5] = gn_w[c0 + 16:c0 + 32]
        cf[0:16, 6] = gn_b[c0:c0 + 16]
        cf[0:16, 7] = gn_b[c0 + 16:c0 + 32]
        m["cstf"] = cf
        in_maps.append(m)
    return in_maps


# ---------------- cached PJRT dispatch (axon path) ----------------

def _get_exec(mdt_name="bfloat16"):
    key = ("exec", mdt_name)
    if key in _CACHE:
        return _CACHE[key]
    import jax
    import jax.numpy as jnp
    from jax.sharding import Mesh, PartitionSpec, NamedSharding
    from jax.experimental.shard_map import shard_map
    from concourse import bass2jax
    import concourse.mybir as mybir

    nc = get_nc(mdt_name)
    bass2jax.install_neuronx_cc_hook()
    partition_name = (nc.partition_id_tensor.name
                      if nc.partition_id_tensor else None)
    in_names, out_names, out_avals, out_np = [], [], [], []
    for alloc in nc.m.functions[0].allocations:
        if not isinstance(alloc, mybir.MemoryLocationSet):
            continue
        name = alloc.memorylocations[0].name
        if alloc.kind == "ExternalInput":
            if name != partition_name:
                in_names.append(name)
        elif alloc.kind == "ExternalOutput":
            shape = tuple(alloc.tensor_shape)
            dtype = mybir.dt.np(alloc.dtype)
            out_names.append(name)
            out_avals.append(jax.core.ShapedArray(shape, dtype))
            out_np.append((shape, dtype))
    n_params = len(in_names)
    n_outs = len(out_names)
    in_names_all = list(in_names) + list(out_names)
    if partition_name is not None:
        in_names_all.append(partition_name)
    donate = tuple(range(n_params, n_params + n_outs))

    def _body(*args):
        operands = list(args)
        if partition_name is not None:
            operands.append(bass2jax.partition_id_tensor())
        outs = bass2jax._bass_exec_p.bind(
            *operands,
            out_avals=tuple(out_avals),
            in_names=tuple(in_names_all),
            out_names=tuple(out_names),
            lowering_input_output_aliases=(),
            sim_require_finite=True,
            sim_require_nnan=True,
            nc=nc,
        )
        return tuple(outs)

    devices = jax.devices()[:NCORES]
    mesh = Mesh(np.asarray(devices), ("core",))
    pspec = PartitionSpec("core")
    in_specs = (pspec,) * (n_params + n_outs)
    out_specs = (pspec,) * n_outs
    sharded = jax.jit(
        shard_map(_body, mesh=mesh, in_specs=in_specs, out_specs=out_specs,
                  check_rep=False),
        donate_argnums=donate, keep_unused=True)
    sh = NamedSharding(mesh, pspec)

    zdtypes = []
    import ml_dtypes
    for s, d in out_np:
        zdtypes.append(jnp.bfloat16 if d == ml_dtypes.bfloat16 else d)

    def _mk_zeros():
        return tuple(
            jnp.zeros((NCORES * s[0],) + tuple(s[1:]), zd)
            for (s, d), zd in zip(out_np, zdtypes))

    zeros_fn = jax.jit(_mk_zeros, out_shardings=(sh,) * n_outs)
    E = dict(sharded=sharded, zeros_fn=zeros_fn, in_names=in_names,
             out_names=out_names, out_np=out_np)
    _CACHE[key] = E
    return E


def _dispatch(in_maps, mdt_name="bfloat16"):
    """concat per-core maps, run on 8 cores, return per-core result dicts.

    Includes host concat + on-device zero-output creation + execute + fetch:
    the same work run_bass_kernel_spmd would do per call, with the jit cached.
    """
    E = _get_exec(mdt_name)
    concat_in = [np.concatenate([m[name] for m in in_maps], axis=0)
                 for name in E["in_names"]]
    zeros = _CACHE.pop(("zeros_next", mdt_name), None) or E["zeros_fn"]()
    out_arrs = E["sharded"](*concat_in, *zeros)
    # prefetch donated buffers for the NEXT call while this one runs
    _CACHE[("zeros_next", mdt_name)] = E["zeros_fn"]()
    # enqueue D2H smallest-first so tiny outputs don't queue behind outv
    order = sorted(range(len(out_arrs)), key=lambda i: out_arrs[i].nbytes)
    for i in order:
        for s_ in out_arrs[i].addressable_shards:
            s_.data.copy_to_host_async()
    outs = [np.asarray(a) for a in out_arrs]
    results = []
    for c in range(NCORES):
        r = {}
        for i, name in enumerate(E["out_names"]):
            s0 = E["out_np"][i][0][0]
            r[name] = outs[i][c * s0:(c + 1) * s0]
        results.append(r)
    return results


def _run_spmd(nc, in_maps):
    """Fallback: stock dispatcher (non-axon environments)."""
    from concourse.bass_utils import run_bass_kernel_spmd
    res = run_bass_kernel_spmd(nc, in_maps, core_ids=list(range(NCORES)))
    return res.results


def kernel(x, w_om, b_om, dcn_w, dcn_b, gn_w, gn_b, offset_scale,
           _mdt="bfloat16"):
    import ml_dtypes

    x = np.asarray(x, np.float32)
    w_om = np.asarray(w_om, np.float32)
    b_om = np.asarray(b_om, np.float32)
    dcn_w = np.asarray(dcn_w, np.float32)
    dcn_b = np.asarray(dcn_b, np.float32)
    gn_w = np.asarray(gn_w, np.float32)
    gn_b = np.asarray(gn_b, np.float32)
    offset_scale = np.asarray(offset_scale, np.float32)
    cast = ml_dtypes.bfloat16 if _mdt == "bfloat16" else np.float32
    in_maps = _host_prep(x, w_om, b_om, dcn_w, dcn_b, gn_w, gn_b,
                         offset_scale, cast)
    try:
        from concourse.bass_utils import axon_active
        use_fast = axon_active()
    except Exception:
        use_fast = False
    if use_fast:
        results = _dispatch(in_maps, _mdt)
    else:
        results = _run_spmd(get_nc(_mdt), in_maps)
    out = np.zeros((4, 64, H, W), np.float32)
    stats = np.zeros((8, 32, 2), np.float32)
    moff_all = 0.0
    for core in range(NCORES):
        b, gp = core // 2, core % 2
        r = results[core]
        out[b, gp * 32:gp * 32 + 16] = np.asarray(
            r["outv0"], np.float32).reshape(16, H, W)
        out[b, gp * 32 + 16:gp * 32 + 32] = np.asarray(
            r["outv1"], np.float32).reshape(16, H, W)
        stats[core] = r["statso"]
        moff_all = max(moff_all, float(np.max(r["moffo"])))
    if moff_all > 1.0:
        out = _host_correct(out, stats, x, w_om, b_om, dcn_w, dcn_b,
                            gn_w, gn_b, offset_scale)
    return out


def _host_correct(out, stats, x, w_om, b_om, dcn_w, dcn_b, gn_w, gn_b,
                  offset_scale):
    """Exact fix for rare pixels with |offset| > 1 (clamped-tri mismatch)."""
    from scipy.special import erf, expit
    sc = float(np.asarray(offset_scale).reshape(-1)[0])
    B = x.shape[0]
    om = (np.einsum('bcp,oc->bop', x.reshape(B, 64, NPIX), w_om)
          + b_om[None, :, None]).reshape(B, 108, H, W)
    for b in range(B):
        for g in range(G):
            oy = om[b, g * 27:g * 27 + 18:2] * sc
            ox = om[b, g * 27 + 1:g * 27 + 18:2] * sc
            bad = (np.abs(oy) > 1).any(0) | (np.abs(ox) > 1).any(0)
            if not bad.any():
                continue
            ml = expit(om[b, g * 27 + 18:g * 27 + 27])
            core = b * 2 + g // 2
            gl = (g % 2) * 16
            N = CG * NPIX
            mu = stats[core, gl:gl + 16, 0].sum() / N
            var = stats[core, gl:gl + 16, 1].sum() / N - mu * mu
            inv = 1.0 / np.sqrt(var + EPS)
            wg = dcn_w[g].reshape(CG, CG, K)
            for hh, ww in zip(*np.nonzero(bad)):
                val = np.zeros((CG, K), np.float32)
                for k in range(K):
                    ky, kx = k // 3, k % 3
                    py = hh + ky - 1 + oy[k, hh, ww]
                    pxx = ww + kx - 1 + ox[k, hh, ww]
                    y0, x0 = int(np.floor(py)), int(np.floor(pxx))
                    fy, fx = py - y0, pxx - x0
                    acc = np.zeros(CG, np.float32)
                    for dy, wy in ((0, 1 - fy), (1, fy)):
                        for dx, wx in ((0, 1 - fx), (1, fx)):
                            yy, xx = y0 + dy, x0 + dx
                            if 0 <= yy < H and 0 <= xx < W:
                                acc += wy * wx * x[b, g * CG:g * CG + CG,
                                                   yy, xx]
                    val[:, k] = acc * ml[k, hh, ww]
                pre = np.einsum('ck,ock->o', val, wg) + dcn_b[g]
                z = ((pre - mu) * inv * gn_w[g * CG:g * CG + CG]
                     + gn_b[g * CG:g * CG + CG])
                out[b, g * CG:g * CG + CG, hh, ww] = (
                    z * 0.5 * (1.0 + erf(z / np.sqrt(2.0))))
    return out
